# revision 96
# baseline (speedup 1.0000x reference)
"""HSTU attention (B=2, L=2048, D=1024, H=16) on 8 TRN2 NeuronCores.

Sharding: batch (2) x head-group (4 heads, 256 features) -> 8 cores.
Host sums the 4 partial W_o outputs per batch.

Design (v2 baseline 182.8us -> 161.2us in the TimelineSim cost model):
  - Split-fp8 projections: the host ships x^T and 64*W^T as e4m3 hi/lo
    residual pairs; Q/K/V/U are 3-term DoubleRow matmuls
    (Wh*xh + Wl*xh + Wh*xl, contraction 256 per pass at 0.5 cyc/row) --
    25% fewer PE cycles than bf16/f32r at ~3e-4 added error. The 64x
    weight scale (keeps fp8 normals) is undone in the exp scale
    (1/(64^2*8), dmask pre-scaled by its inverse on the host) and in the
    V/U store tensor_scalar_mul. S stays f32r (split-fp8 S would need 3
    DR terms at 768 > 512 cyc/pair: a loss).
  - S^T tiles for the 2 heads of an ec in one 2-bank PSUM tile
    [128, 2x512]; one exp per (ec, jc, ic), bias as a per-partition
    column, bf16 out. Diagonal pairs split the exp at the diagonal
    subtile; only that subtile gets a 2D mask add (DVE; gpsimd cannot
    reach PSUM -- breaks neuronxcc codegen).
  - S runs ONE PAIR AHEAD of exp: per pair the PE stream is
    [AV(p-5)] [S(p+1)] [paced fillers] [exp(p) on ACT], so filler
    overshoot can never delay the next exp (ACT exp, 1038ns/pair, is the
    in-loop bottleneck; PE total 120.8us vs ACT 117.4us).
  - Fillers are a global deadline-aware queue of GENERATORS yielding
    ~400-1000-cycle chunks (only the head generator is in flight, so the
    shared PSUM pools stay serial): K/V of later blocks with per-pair
    deadlines, U(ic) riding ic+1's queue ahead of the gate generator
    (ic+1's loops are ACT-bound with PE idle to absorb it; ic's own tail
    backlog would starve ACT across the boundary instead), Q(ic+1) due 4
    pairs before the boundary, W_o(ic-1) spilling freely into later ics.
    Pacing rate = max(1000 cyc/pair, deadline-bound work / pairs). The AV
    pipeline depth tapers 5 -> 0 over each ic's last pairs so the final
    flush shrinks and avs staging + gating start earlier.
  - av PSUM accumulates via one start=True per bank (zero-region) and
    stop=True on the final AV matmuls (no 1.6us DVE memset); av staged
    to SBUF bf16 per bank, with per-bank reciprocals so gating starts
    after bank 0's staging; gating is a two-phase generator (all DVE stt first,
    PE transposes after) paced into the next ic so the in-order PE queue
    never blocks on DVE at ic boundaries. Final ic fuses gating+W_o with
    output copies split ACT/DVE.
  - seq_lens-aware: njc = ceil(max(seq_lens)/128) key tiles (compile
    cache keyed on njc); K projection of the last partial block trims its
    moving columns. Startup interleaves the wk/x0/wq DMAs with the K0
    matmul chunks (first Ldweights ~1.5us in); the startup x loads ride
    the Activation-engine HWDGE queue, parallel to the weight loads on
    the SP queue (in-loop x loads stay on SP so they never queue ahead
    of exps).
"""

import sys

for _p in ("/opt/trn_rl_repo", "/root/.axon_site/_ro/trn_rl_repo"):
    if _p not in sys.path:
        sys.path.insert(0, _p)

import ml_dtypes
import numpy as np

import concourse.bass as bass  # noqa: F401
import concourse.mybir as mybir
import concourse.tile as tile
from concourse import bacc
from concourse.bass_utils import run_bass_kernel_spmd

F32 = mybir.dt.float32
F32R = mybir.dt.float32r
BF16 = mybir.dt.bfloat16
F8 = mybir.dt.float8e4
EXP = mybir.ActivationFunctionType.Exp
MULT = mybir.AluOpType.mult
DRM = mybir.MatmulPerfMode.DoubleRow

B, L, D, H = 2, 2048, 1024, 16
DK = D // H          # 64
HPC = 4              # heads per core
E = HPC * DK         # 256 features per core
NDC = D // 128       # 8 contraction chunks for projections
NIC = L // 512       # 4 query blocks
NEG = -10000.0
WSC = 64.0           # host scale on W_{q,k,v,u} so fp8 hi/lo spans normals
SEXP = 1.0 / (WSC * WSC * 8.0)   # exp scale: undo 64^2 and sqrt(dk)=8

_cache = {}


def _slot(k):
    """PSUM offset (f32 words) of av slot k: 7 slots of 65 per 2KB bank."""
    return (k // 7) * 512 + (k % 7) * 65


def _build(njc):
    nc = bacc.Bacc("TRN2", target_bir_lowering=False, debug=False)

    xth = nc.dram_tensor("xth", [D, L], F8, kind="ExternalInput").ap()
    xtl = nc.dram_tensor("xtl", [D, L], F8, kind="ExternalInput").ap()
    wpairs = {
        w: tuple(
            nc.dram_tensor(f"w{w}{s}", [D, E], F8, kind="ExternalInput").ap()
            for s in ("h", "l")
        )
        for w in ("q", "k", "v", "u")
    }
    wo = nc.dram_tensor("wo", [E, D], BF16, kind="ExternalInput").ap()
    biasab = nc.dram_tensor("biasab", [128, njc], F32, kind="ExternalInput").ap()
    biasbl = nc.dram_tensor("biasbl", [128, njc], F32, kind="ExternalInput").ap()
    dmask = nc.dram_tensor("dmask", [njc, 128, 128], BF16, kind="ExternalInput").ap()
    ident = nc.dram_tensor("ident", [128, 128], BF16, kind="ExternalInput").ap()
    out = nc.dram_tensor("out", [L, D], BF16, kind="ExternalOutput").ap()

    with tile.TileContext(nc) as tc:
        with tc.tile_pool(name="persist", bufs=1) as P:
            kt = [P.tile([128, L], F32R, tag=f"kt{i}", name=f"kt{i}") for i in range(2)]
            qt = [P.tile([128, L], F32R, tag=f"qt{i}", name=f"qt{i}") for i in range(2)]
            ut = P.tile([128, 16, E], BF16, tag="ut", name="ut")
            vt = [
                P.tile([128, njc, 2, 65], BF16, tag=f"vt{i}", name=f"vt{i}")
                for i in range(2)
            ]
            gt = {
                (ec, lc): P.tile(
                    [128, 128], BF16, tag=f"gt{ec}_{lc}", name=f"gt{ec}_{lc}"
                )
                for ec in range(2)
                for lc in range(16)
            }
            wts = {
                w: tuple(
                    P.tile([128, NDC, E], F8, tag=f"w{w}{s}", name=f"w{w}{s}")
                    for s in ("h", "l")
                )
                for w in ("q", "k", "v", "u")
            }
            wor = P.tile([128, 2, D], BF16, tag="wor", name="wor")
            bab = P.tile([128, njc], F32, tag="bab", name="bab")
            bbl = P.tile([128, njc], F32, tag="bbl", name="bbl")
            dmt = P.tile([128, njc, 128], BF16, tag="dmt", name="dmt")
            idt = P.tile([128, 128], BF16, tag="idt", name="idt")

            # ones columns of vt (slot 64 of each [V|1] group)
            for ec in range(2):
                nc.vector.memset(vt[ec][:, :, :, 64:65], 1.0)

            def _wload(w, s, lohalf=None):
                src = wpairs[w][s]
                dst = wts[w][s]
                if lohalf is None:
                    nc.sync.dma_start(
                        out=dst, in_=src.rearrange("(dc p) e -> p dc e", p=128)
                    )
                else:
                    half = NDC // 2
                    d0, d1 = (0, half) if lohalf == 0 else (half, NDC)
                    nc.sync.dma_start(
                        out=dst[:, d0:d1, :],
                        in_=src[d0 * 128 : d1 * 128, :].rearrange(
                            "(dc p) e -> p dc e", p=128
                        ),
                    )

            def preload_early():
                _wload("k", 0, 0)
                _wload("k", 1, 0)

            def preload_early2():
                _wload("k", 0, 1)
                _wload("k", 1, 1)
                nc.sync.dma_start(out=bab, in_=biasab)
                nc.sync.dma_start(out=bbl, in_=biasbl)
                nd = min(4, njc)
                nc.sync.dma_start(
                    out=dmt[:, 0:nd], in_=dmask[0:nd].rearrange("j p i -> p j i")
                )
                _wload("v", 0)
                _wload("v", 1)
                nc.sync.dma_start(out=idt, in_=ident)

            def preload_mid():
                _wload("q", 0)
                _wload("q", 1)
                _wload("u", 0)
                _wload("u", 1)

            def preload_late():
                if njc > 4:
                    nc.sync.dma_start(
                        out=dmt[:, 4:njc],
                        in_=dmask[4:njc].rearrange("j p i -> p j i"),
                    )
                nc.sync.dma_start(
                    out=wor, in_=wo.rearrange("(ec p) d -> p ec d", p=128)
                )

            with tc.tile_pool(name="xa", bufs=4) as xap, \
                 tc.tile_pool(name="epool", bufs=8) as epool, \
                 tc.tile_pool(name="gpool", bufs=8) as gpool, \
                 tc.tile_pool(name="avs", bufs=3) as avsp, \
                 tc.tile_pool(name="rpool", bufs=4) as rpool, \
                 tc.tile_pool(name="ostg", bufs=4) as ostg, \
                 tc.tile_pool(name="ps_s", bufs=2, space="PSUM") as ps_s, \
                 tc.tile_pool(name="ps_av", bufs=1, space="PSUM") as ps_av, \
                 tc.tile_pool(name="ps_pw", bufs=1, space="PSUM") as ps_pw:

                def load_x(ic):
                    t = xap.tile([128, 2, NDC, 512], F8, tag="x", name="xa")
                    for hl, src in enumerate((xth, xtl)):
                        nc.sync.dma_start(
                            out=t[:, hl, :, :],
                            in_=src[:, ic * 512 : (ic + 1) * 512].rearrange(
                                "(dc p) i -> p dc i", p=128
                            ),
                        )
                    return t

                def gen_kq(xa_t, ic, ec, wsrc, dst, pool=None, ptag=None,
                           ncol=512, col0=0):
                    # 3-term split-fp8: Wh*xh + Wl*xh + Wh*xl, DoubleRow over
                    # dc pairs. PSUM holds 64*K (or 64*Q); descale in exp.
                    # Yields per-term chunks so pacing stays fine-grained; the
                    # xl-dependent term goes last so the first block can start
                    # before the x-lo DMA lands.
                    isl = slice(ic * 512 + col0, ic * 512 + col0 + ncol)
                    esl = slice(ec * 128, (ec + 1) * 128)
                    p = (pool or ps_pw).tile(
                        [128, 512], F32, tag=ptag or "pw", name="pkq"
                    )
                    po = p[:, col0 : col0 + ncol]
                    nt = 3 * (NDC // 2)
                    i = 0
                    for ws, xs in ((0, 0), (1, 0), (0, 1)):
                        for dc2 in range(NDC // 2):
                            sl2 = slice(2 * dc2, 2 * dc2 + 2)
                            nc.tensor.matmul(
                                po,
                                wsrc[ws][:, sl2, esl],
                                xa_t[:, xs, sl2, col0 : col0 + ncol],
                                start=(i == 0),
                                stop=(i == nt - 1),
                                perf_mode=DRM,
                            )
                            i += 1
                        yield ncol * 2
                    nc.vector.tensor_copy(dst[ec][:, isl], po)

                def emit_kq(*a, **kw):
                    for _ in gen_kq(*a, **kw):
                        pass

                def gen_vu(w, xa_t, ic, it, pool=None, ptag=None):
                    lc = 4 * ic + it
                    tsl = slice(it * 128, (it + 1) * 128)
                    p = (pool or ps_pw).tile(
                        [128, 512], F32, tag=ptag or "pw", name=f"p{w}"
                    )
                    pv = p[:, 0:E]
                    nt = 3 * (NDC // 2)
                    i = 0
                    for xs, ws in ((0, 0), (1, 0), (0, 1)):
                        for dc2 in range(NDC // 2):
                            sl2 = slice(2 * dc2, 2 * dc2 + 2)
                            nc.tensor.matmul(
                                pv,
                                xa_t[:, xs, sl2, tsl],
                                wts[w][ws][:, sl2, :],
                                start=(i == 0),
                                stop=(i == nt - 1),
                                perf_mode=DRM,
                            )
                            i += 1
                        yield 512
                    if w == "v":
                        with nc.allow_low_precision(reason="bf16 V"):
                            for ec in range(2):
                                nc.vector.tensor_scalar_mul(
                                    vt[ec][:, lc, :, 0:64],
                                    pv[:, ec * 128 : (ec + 1) * 128].rearrange(
                                        "p (h v) -> p h v", h=2
                                    ),
                                    1.0 / WSC,
                                )
                    else:
                        with nc.allow_low_precision(reason="bf16 U"):
                            nc.vector.tensor_scalar_mul(
                                ut[:, lc, :], pv, 1.0 / WSC
                            )

                def emit_v(xa_t, ic, it, pool=None, ptag=None):
                    for _ in gen_vu("v", xa_t, ic, it, pool, ptag):
                        pass

                ostate = {}

                def emit_wo(ic, it, fc, pool=None, ptag=None, copier=None):
                    lc = 4 * ic + it
                    if fc == 0:
                        ostate[lc] = ostg.tile([128, 1024], BF16, tag="o", name="ostg")
                    p = (pool or ps_pw).tile(
                        [128, 512], F32, tag=ptag or "pw", name="pwo"
                    )
                    for ec in range(2):
                        nc.tensor.matmul(
                            p,
                            gt[(ec, lc)],
                            wor[:, ec, fc * 512 : (fc + 1) * 512],
                            start=(ec == 0),
                            stop=(ec == 1),
                        )
                    o = ostate[lc]
                    cp = copier or nc.vector
                    with nc.allow_low_precision(reason="bf16 output"):
                        if cp is nc.scalar:
                            cp.copy(o[:, fc * 512 : (fc + 1) * 512], p)
                        else:
                            cp.tensor_copy(o[:, fc * 512 : (fc + 1) * 512], p)
                    if fc == 1:
                        nc.sync.dma_start(
                            out=out[lc * 128 : (lc + 1) * 128, :], in_=o
                        )

                def gen_wo(ic, it, fc):
                    emit_wo(ic, it, fc)
                    yield 1024

                def kcols(bk):
                    return max(0, min(512, njc * 128 - bk * 512))

                # ---------- upfront: K and Q for block 0 / ic=0; everything
                # else is deadline-paced filler inside the attention loops.
                # DMAs are interleaved with the K0 matmul chunks so the first
                # matmul only waits on wk-hi half0 + x0-hi half0. ----------
                nkb = -(-njc // 4)  # 512-blocks of keys needed
                half = NDC // 2
                xs = {}
                x0 = xap.tile([128, 2, NDC, 512], F8, tag="x", name="xa")
                xs[0] = x0

                def x0load(hl, dh):
                    src = (xth, xtl)[hl]
                    nc.scalar.dma_start(
                        out=x0[:, hl, dh * half : (dh + 1) * half, :],
                        in_=src[dh * half * 128 : (dh + 1) * half * 128, 0:512]
                        .rearrange("(dc p) i -> p dc i", p=128),
                    )

                _wload("k", 0, 0)
                x0load(0, 0)
                _wload("k", 0, 1)
                x0load(0, 1)
                _wload("k", 1, 0)
                _wload("k", 1, 1)
                x0load(1, 0)
                x0load(1, 1)
                # K0 both ec, chunk-interleaved with the Q weight loads
                gk = [gen_kq(xs[0], 0, ec, wts["k"], kt, pool=ps_s, ptag="s",
                             ncol=kcols(0)) for ec in range(2)]
                next(gk[0]); next(gk[1])
                _wload("q", 0)
                _wload("q", 1)
                next(gk[0]); next(gk[1])
                nc.sync.dma_start(out=bab, in_=biasab)
                nc.sync.dma_start(out=bbl, in_=biasbl)
                nd = min(4, njc)
                nc.sync.dma_start(
                    out=dmt[:, 0:nd], in_=dmask[0:nd].rearrange("j p i -> p j i")
                )
                # finish ec0's K and Q first: S(pair 1) only needs them
                for _ in gk[0]:
                    pass
                emit_kq(xs[0], 0, 0, wts["q"], qt, pool=ps_s, ptag="s")
                _wload("v", 0)
                _wload("v", 1)
                nc.sync.dma_start(out=idt, in_=ident)
                for _ in gk[1]:
                    pass
                emit_kq(xs[0], 0, 1, wts["q"], qt, pool=ps_s, ptag="s")
                xs[1] = load_x(1)
                _wload("u", 0)
                _wload("u", 1)
                xs[2] = load_x(2)
                preload_late()
                xs[3] = load_x(3)

                def gen_gate(ic, avs):
                    # paced gating for ic, two-phase: all DVE gating first
                    # (cheap on PE), then the PE transposes — by the time a
                    # transpose pops, its gating input is long done, so the
                    # in-order PE queue never blocks on DVE
                    avsr = avs.rearrange("p (s c) -> p s c", c=65)
                    rec = rpool.tile([128, 16], F32, tag="rec", name="rec")
                    with nc.allow_low_precision(reason="softmax recip"):
                        # per-bank recips: gating of the first slots starts
                        # after bank 0's staging instead of all three banks
                        for r0, r1 in ((0, 7), (7, 14), (14, 16)):
                            nc.vector.reciprocal(
                                rec[:, r0:r1], avsr[:, r0:r1, 64]
                            )
                    gs = {}
                    for it in range(4):
                        lc = 4 * ic + it
                        for ec in range(2):
                            g = gpool.tile([128, 128], BF16, tag="g", name="g")
                            gs[(ec, it)] = g
                            with nc.allow_low_precision(reason="bf16 gating"):
                                for h in range(2):
                                    grp = 2 * ec + h
                                    k = it * 4 + grp
                                    nc.vector.scalar_tensor_tensor(
                                        g[:, h * 64 : (h + 1) * 64],
                                        avsr[:, k, 0:64],
                                        rec[:, k : k + 1],
                                        ut[:, lc, grp * 64 : (grp + 1) * 64],
                                        MULT,
                                        MULT,
                                    )
                            yield 64
                    for it in range(4):
                        lc = 4 * ic + it
                        for ec in range(2):
                            pt = ps_pw.tile([128, 128], BF16, tag="pw", name="pt")
                            nc.tensor.transpose(pt, gs[(ec, it)], idt)
                            with nc.allow_low_precision(reason="bf16 gt"):
                                nc.vector.tensor_copy(gt[(ec, lc)], pt)
                            yield 512

                def gate_ic(ic, avs, fuse_wo=False):
                    avsr = avs.rearrange("p (s c) -> p s c", c=65)
                    rec = rpool.tile([128, 16], F32, tag="rec", name="rec")
                    with nc.allow_low_precision(reason="softmax recip"):
                        for r0, r1 in ((0, 7), (7, 14), (14, 16)):
                            nc.vector.reciprocal(
                                rec[:, r0:r1], avsr[:, r0:r1, 64]
                            )
                    for it in range(4):
                        lc = 4 * ic + it
                        for ec in range(2):
                            g = gpool.tile([128, 128], BF16, tag="g", name="g")
                            with nc.allow_low_precision(reason="bf16 gating"):
                                for h in range(2):
                                    grp = 2 * ec + h
                                    k = it * 4 + grp
                                    nc.vector.scalar_tensor_tensor(
                                        g[:, h * 64 : (h + 1) * 64],
                                        avsr[:, k, 0:64],
                                        rec[:, k : k + 1],
                                        ut[:, lc, grp * 64 : (grp + 1) * 64],
                                        MULT,
                                        MULT,
                                    )
                            pt = (ps_s if fuse_wo else ps_pw).tile(
                                [128, 128], BF16,
                                tag="s" if fuse_wo else "pw", name="pt"
                            )
                            nc.tensor.transpose(pt, g, idt)
                            with nc.allow_low_precision(reason="bf16 gt"):
                                if fuse_wo:
                                    nc.scalar.copy(gt[(ec, lc)], pt)
                                else:
                                    nc.vector.tensor_copy(gt[(ec, lc)], pt)
                        if fuse_wo:
                            for fc in range(2):
                                emit_wo(
                                    ic, it, fc, pool=ps_s, ptag="s",
                                    copier=nc.scalar if fc == 0 else nc.vector,
                                )

                # ---------- main loop ----------
                # Global filler queue of (weight_est, generator, deadline).
                # Generators yield per-chunk PE-cycle weights; only the head
                # generator is in flight (keeps shared PSUM pools serial).
                # deadline d >= 0: all chunks must run before pair d of the
                # current ic; d == -1: flush at end of the current ic; None:
                # may spill into later ics.
                fq = []
                pend_gate = None
                for ic in range(NIC):
                    if ic == 0:
                        for it in range(4):
                            fq.append((1536, gen_vu("v", xs[0], 0, it),
                                       4 + it))
                        for b in range(1, nkb):
                            for ec in range(2):
                                fq.append((6 * kcols(b),
                                           gen_kq(xs[b], b, ec, wts["k"], kt,
                                                  ncol=kcols(b)),
                                           max(1, 8 * b - 1)))
                            for it in range(4):
                                if 4 * b + it < njc:
                                    fq.append((1536, gen_vu("v", xs[b], b, it),
                                               8 * b + 4 + it))
                    if ic > 0:
                        for it in range(4):
                            for fc in range(2):
                                fq.append((1024, gen_wo(ic - 1, it, fc), None))
                    # U(NIC-1) is needed by the final fused gate right after
                    # the loop; earlier U(ic) rides ic+1's queue (ahead of the
                    # gate generator) where PE has idle to absorb it.
                    if ic == NIC - 1:
                        for it in range(4):
                            fq.append((1536, gen_vu("u", xs[ic], ic, it),
                                       2 * njc - 2))
                    if ic + 1 < NIC:
                        for ec in range(2):
                            fq.append((3072,
                                       gen_kq(xs[ic + 1], ic + 1, ec,
                                              wts["q"], qt),
                                       2 * njc - 4))

                    av = ps_av.tile([128, 1536], F32, tag="av", name="av")
                    # per-bank start=True zeroes each bank's region; the final
                    # AV matmuls carry stop=True so next ic can start again
                    av_banks = set()
                    if pend_gate is not None:
                        fq.insert(0, (4096, gen_gate(*pend_gate), -1))
                        pend_gate = None
                        for it in reversed(range(4)):
                            fq.insert(0, (1536,
                                          gen_vu("u", xs[ic - 1], ic - 1, it),
                                          10 + 2 * it))

                    isl = slice(ic * 512, (ic + 1) * 512)
                    npair = 2 * njc
                    pair = 0
                    wdone = 0
                    pend_av = []
                    # filler cycles per pair: at least the ACT-vs-PE deficit,
                    # more if this ic's deadline-bound work demands it
                    wbound = sum(w for w, _g, d in fq if d is not None)
                    pace = max(1000, wbound // npair + 16)

                    def drain(g):
                        n = 0
                        for w in g:
                            n += w
                        return n

                    def pump(g):
                        return next(g, None)

                    def emit_av(e, jc, ec, stops=frozenset()):
                        for h in range(2):
                            grp = 2 * ec + h
                            for it in range(4):
                                k = it * 4 + grp
                                bank = k // 7
                                st = bank not in av_banks
                                av_banks.add(bank)
                                nc.tensor.matmul(
                                    av[:, _slot(k) : _slot(k) + 65],
                                    e[:, h, it, :],
                                    vt[ec][:, jc, h, :],
                                    start=st,
                                    stop=(k in stops),
                                    skip_group_check=True,
                                )

                    def emit_s(jc, ec):
                        # S matmuls + (diagonal) mask add on GPSIMD, which is
                        # idle; keeping the add off DVE avoids stalling exp
                        # behind the DVE backlog
                        jsl = slice(jc * 128, (jc + 1) * 128)
                        stile = ps_s.tile([128, 1024], F32, tag="s",
                                          name="stile")
                        for h in range(2):
                            nc.tensor.matmul(
                                stile[:, h * 512 : (h + 1) * 512],
                                kt[ec][h * 64 : (h + 1) * 64, jsl],
                                qt[ec][h * 64 : (h + 1) * 64, isl],
                                start=True,
                                stop=True,
                            )
                        if jc // 4 == ic:
                            sv = stile.rearrange("p (h i) -> p h i", h=2)
                            t0 = jc % 4
                            w0 = t0 * 128
                            nc.vector.tensor_add(
                                sv[:, :, w0 : w0 + 128],
                                sv[:, :, w0 : w0 + 128],
                                dmt[:, jc, :].unsqueeze(1).broadcast_to(
                                    [128, 2, 128]
                                ),
                            )
                        return stile

                    def emit_exp(stile, jc, ec):
                        e = epool.tile([128, 2, 4, 128], BF16, tag="e", name="e")
                        sv = stile.rearrange("p (h i) -> p h i", h=2)
                        if jc // 4 == ic:
                            # diagonal pair: columns below the diagonal
                            # subtile see the prompt-only (ab) bias; the
                            # diagonal subtile (2D mask added on gpsimd) and
                            # columns above use the valid-only (bl) bias
                            t0 = jc % 4
                            w0 = t0 * 128
                            with nc.allow_low_precision(reason="bf16 softmax"):
                                if t0 > 0:
                                    nc.scalar.activation(
                                        e[:, :, 0:t0, :],
                                        sv[:, :, 0:w0].rearrange(
                                            "p h (it i) -> p h it i", i=128
                                        ),
                                        EXP,
                                        bias=bab[:, jc : jc + 1],
                                        scale=SEXP,
                                    )
                                nc.scalar.activation(
                                    e[:, :, t0:4, :],
                                    sv[:, :, w0:512].rearrange(
                                        "p h (it i) -> p h it i", i=128
                                    ),
                                    EXP,
                                    bias=bbl[:, jc : jc + 1],
                                    scale=SEXP,
                                )
                        else:
                            bias = bab if jc // 4 > ic else bbl
                            with nc.allow_low_precision(reason="bf16 softmax"):
                                nc.scalar.activation(
                                    e.rearrange("p h it i -> p (h it i)"),
                                    stile,
                                    EXP,
                                    bias=bias[:, jc : jc + 1],
                                    scale=SEXP,
                                )
                        pend_av.append((e, jc, ec))

                    # S runs one pair ahead of exp so filler overshoot never
                    # delays the next exp (ACT is the in-loop bottleneck)
                    pairs = [(jc, ec) for jc in range(njc) for ec in range(2)]
                    pend_s = emit_s(*pairs[0])
                    for pidx, (jc, ec) in enumerate(pairs):
                        cur = pend_s
                        depth = min(5, len(pairs) - 1 - pidx)
                        while len(pend_av) > depth:
                            emit_av(*pend_av.pop(0))
                        if pidx + 1 < len(pairs):
                            pend_s = emit_s(*pairs[pidx + 1])
                        emit_exp(cur, jc, ec)
                        # paced filler work, deadline-aware
                        pair += 1
                        due = None
                        for fi, (_w, _g, _d) in enumerate(fq):
                            if _d is not None and 0 <= _d <= pair:
                                due = fi
                        if due is not None:
                            for _w, _g, _d in fq[: due + 1]:
                                wdone += drain(_g)
                            del fq[: due + 1]
                        while fq and wdone < pair * pace:
                            got = pump(fq[0][1])
                            if got is None:
                                fq.pop(0)
                            else:
                                wdone += got
                    for pi, pa in enumerate(pend_av):
                        if pi == len(pend_av) - 1:
                            # last AV call (always ec=1): close all 3 bank
                            # groups on each bank's final matmul
                            emit_av(*pa, stops=frozenset((3, 11, 15)))
                        else:
                            emit_av(*pa)
                    # flush ic-bound fillers (U of ic, Q of ic+1); spill the
                    # rest (W_o) into later loops, everything on the last ic
                    keep = []
                    for w, g, d in fq:
                        if d == -1 or ic == NIC - 1:
                            drain(g)
                        else:
                            keep.append((w, g, d))
                    fq = keep
                    # stage av to SBUF (bf16) so gating doesn't block next memset
                    avs = avsp.tile([128, 16 * 65], BF16, tag="avs", name="avs")
                    with nc.allow_low_precision(reason="bf16 av staging"):
                        for r in range(3):
                            cnt = min(7, 16 - 7 * r)
                            nc.vector.tensor_copy(
                                avs[:, 7 * r * 65 : (7 * r + cnt) * 65],
                                av[:, r * 512 : r * 512 + cnt * 65],
                            )
                    pend_gate = (ic, avs)

                gate_ic(*pend_gate, fuse_wo=True)

    nc.compile()
    return nc


def _split8(t):
    """Split f32 array into (hi, lo) e4m3 pair with hi+lo ~ t (rel ~3e-4)."""
    t = np.asarray(t, np.float32)
    hi = t.astype(ml_dtypes.float8_e4m3)
    lo = (t - hi.astype(np.float32)).astype(ml_dtypes.float8_e4m3)
    return np.ascontiguousarray(hi), np.ascontiguousarray(lo)


def _host_inputs(njc, x, token_types, seq_lens, W_q, W_k, W_v, W_u, W_o):
    x = np.asarray(x, dtype=np.float32)
    token_types = np.asarray(token_types)
    seq_lens = np.asarray(seq_lens)
    W_q = np.asarray(W_q, dtype=np.float32)
    W_k = np.asarray(W_k, dtype=np.float32)
    W_v = np.asarray(W_v, dtype=np.float32)
    W_u = np.asarray(W_u, dtype=np.float32)
    W_o = np.asarray(W_o, dtype=np.float32)

    per_batch = []
    jr = np.arange(L)
    for b in range(B):
        xh, xl = _split8(x[b].T)
        prompt = token_types[b] < 3
        valid = jr < int(seq_lens[b])
        ab = np.where(prompt & valid, 0.0, NEG).astype(np.float32)
        bl = np.where(valid, 0.0, NEG).astype(np.float32)
        biasab = np.ascontiguousarray(ab[: njc * 128].reshape(njc, 128).T)
        biasbl = np.ascontiguousarray(bl[: njc * 128].reshape(njc, 128).T)
        # diagonal-tile 2D masks, on top of the bl bias: NEG only where a
        # valid item key sits above the diagonal. Added to scaled scores
        # (before the exp descale), so pre-multiply by 1/SEXP.
        dmk = np.empty((njc, 128, 128), np.float32)
        for jc in range(njc):
            j = jr[jc * 128 : (jc + 1) * 128]
            i = np.arange(jc * 128, (jc + 1) * 128)  # diagonal subtile only
            allowed = valid[j][:, None] & (
                prompt[j][:, None] | (j[:, None] <= i[None, :])
            )
            full = np.where(allowed, 0.0, NEG).astype(np.float32)
            dmk[jc] = (full - bl[j][:, None]) / SEXP
        per_batch.append((xh, xl, biasab, biasbl, dmk))

    in_maps = []
    for c in range(8):
        b, gi = c // 4, c % 4
        e0 = E * gi
        xh, xl, biasab, biasbl, dmk = per_batch[b]
        m = {
            "xth": xh,
            "xtl": xl,
            "wo": np.ascontiguousarray(W_o[:, e0 : e0 + E].T).astype(
                ml_dtypes.bfloat16
            ),
            "ident": np.eye(128, dtype=np.float32).astype(ml_dtypes.bfloat16),
            "biasab": biasab,
            "biasbl": biasbl,
            "dmask": dmk.astype(ml_dtypes.bfloat16),
        }
        for w, W in (("q", W_q), ("k", W_k), ("v", W_v), ("u", W_u)):
            hi, lo = _split8(W[e0 : e0 + E].T * WSC)
            m[f"w{w}h"] = hi
            m[f"w{w}l"] = lo
        in_maps.append(m)
    return in_maps


def kernel(x, token_types, seq_lens, W_q, W_k, W_v, W_u, W_o, **_run_kwargs):
    seq_lens = np.asarray(seq_lens)
    njc = int(-(-int(seq_lens.max()) // 128))
    njc = max(1, min(njc, L // 128))
    if ("nc", njc) not in _cache:
        _cache[("nc", njc)] = _build(njc)
    nc = _cache[("nc", njc)]
    _cache["nc"] = nc
    in_maps = _host_inputs(njc, x, token_types, seq_lens, W_q, W_k, W_v, W_u, W_o)
    def _run():
        res = run_bass_kernel_spmd(
            nc, in_maps, list(range(8)), **_run_kwargs
        )
        # materialize inside the retry scope: results are lazy jax arrays,
        # so a device wedge can otherwise surface after the call returns
        outs = [np.asarray(res.results[c]["out"]) for c in range(8)]
        return res, outs

    try:
        res, outs = _run()
    except Exception as ex:  # transient NRT device wedge: retry once
        if "UNRECOVERABLE" not in str(ex) and "UNAVAILABLE" not in str(ex):
            raise
        res, outs = _run()
    _cache["last_result"] = res
    full = np.zeros((B, L, D), np.float64)
    for c in range(8):
        full[c // 4] += outs[c].astype(np.float64)
    return full.astype(np.float32)

